# revision 1
# baseline (speedup 1.0000x reference)
"""CharCNN word encoder on 8 Trainium2 cores.

Strategy (pure data parallelism over valid words):
  * Host: compact valid words (words_mask), compute per-word needed position
    count L (last valid char position + 1), sort words by L, group into
    128-word blocks, stripe blocks across the 8 cores so every core gets an
    identical per-block Lmax schedule (SPMD: one NEFF for all cores).
  * Host embeds chars (262x8 table) and lays each shard out as two matmul
    stationary operands xa/xb [117, nwords]:
        rows  0..95 : embedded chars (seg A: char positions 0..11,
                      seg B: positions 8..19), row p*8+i = emb[chars[w,p], i]
        rows 96..115: char-invalid indicator rows (1.0 where masked)
        row  116    : ones (bias row)
  * Host builds constant Toeplitz matrices ta/tb [117, 10*150] (c-major
    columns: col = c_local*150 + o) encoding the three convs (k=3,4,5, 50
    out-ch each, 'same' pad), the -1e5 mask penalty, and the bias.
  * Device, per block (Lmax = L̂): psum[words, (c,o)] = x_blk.T @ t[:, :N]
    with N = L̂-dependent column count (fp32r matmuls at full PE rate,
    512-aligned chunks; mask/bias folded into extra contraction rows).
    VectorE max-reduces each segment from PSUM in one strided op; a TT-max
    merges the two segments. Blocks run longest-first (input DMA hides under
    their reduces; a few tiny blocks are rotated to the front as warm-up),
    and trailing small blocks batch their outputs into 4-block strips to cut
    DMA-issue serialization at the kernel tail.
  * Host: un-permute shard outputs and apply the words_id gather.
"""

import os
import sys

if "/opt/trn_rl_repo" not in sys.path:
    sys.path.insert(0, "/opt/trn_rl_repo")
# The bass->PJRT path needs the axon jax platform; undo any cpu pinning.
if os.environ.get("JAX_PLATFORMS") == "cpu":
    del os.environ["JAX_PLATFORMS"]

import numpy as np

_KS = (3, 4, 5)
_OC = 50
_NOUT = 150
_NEG = -100000.0
_NCORES = 8
_BLK = 128
_CA = 10                 # c-positions per segment
_NCOLS = _NOUT * _CA     # 1500
_KROWS = 117
_C = 20

_programs: dict = {}
_last_run = None
# bf16 tree leg: ScalarE drains part of PSUM to bf16 SBUF, VectorE max-trees
# it at 2x rate. ~20% faster, costs ~10x on relative error (2e-4 -> ~1.5e-3).
_BF16_TREE = os.environ.get("KERNEL_BF16_TREE", "0") == "1"


def _build_toeplitz(ws, bs):
    """ta, tb: [117, 1500] f32, c-major columns (col = c_local*150 + o)."""
    out = []
    for p_base, c_base in ((0, 0), (8, 10)):
        t = np.zeros((_KROWS, _NCOLS), np.float32)
        for o in range(_NOUT):
            k = _KS[o // _OC]
            oo = o % _OC
            w = ws[k]
            off = k // 2
            for cl in range(_CA):
                c = c_base + cl
                col = cl * _NOUT + o
                for pl in range(12):
                    p = p_base + pl
                    dk = p - c + off
                    if 0 <= dk < k:
                        t[pl * 8:(pl + 1) * 8, col] = w[oo, :, dk]
                t[96 + c, col] = _NEG
                t[116, col] = bs[k][oo]
        out.append(t)
    return out


def _build_x(chars, cmask, emb):
    """xa, xb: [117, n] f32 for one shard."""
    n = chars.shape[0]
    x = emb[np.clip(chars, 0, emb.shape[0] - 1)]        # [n, 20, 8]
    xr = np.ascontiguousarray(x.transpose(1, 2, 0)).reshape(20 * 8, n)
    inv = (~cmask).T.astype(np.float32)                  # [20, n]
    ones = np.ones((1, n), np.float32)
    xa = np.concatenate([xr[0:96], inv, ones], axis=0)
    xb = np.concatenate([xr[64:160], inv, ones], axis=0)
    return np.ascontiguousarray(xa), np.ascontiguousarray(xb)


def _chunks512(n):
    """512-aligned matmul column chunks covering [0, n)."""
    out = []
    c0 = 0
    while c0 < n:
        out.append((c0, min(512, n - c0)))
        c0 += 512
    return out


def _get_program(schedule):
    """schedule: tuple of per-block Lmax values (same for every core)."""
    key = (schedule, _BF16_TREE)
    if key in _programs:
        return _programs[key]

    from contextlib import ExitStack

    import concourse.bacc as bacc
    import concourse.mybir as mybir
    import concourse.tile as tile

    nblocks = len(schedule)
    nwords = nblocks * _BLK
    f32 = mybir.dt.float32
    f32r = mybir.dt.float32r
    bf16 = mybir.dt.bfloat16
    X = mybir.AxisListType.X
    MAX = mybir.AluOpType.max

    nc = bacc.Bacc("TRN2", target_bir_lowering=False, debug=False)
    xa_d = nc.dram_tensor("xa", [_KROWS, nwords], f32r, kind="ExternalInput").ap()
    xb_d = nc.dram_tensor("xb", [_KROWS, nwords], f32r, kind="ExternalInput").ap()
    ta_d = nc.dram_tensor("ta", [_KROWS, _NCOLS], f32r, kind="ExternalInput").ap()
    tb_d = nc.dram_tensor("tb", [_KROWS, _NCOLS], f32r, kind="ExternalInput").ap()
    feat_d = nc.dram_tensor("feat", [nwords, _NOUT], f32, kind="ExternalOutput").ap()

    DMA_CHUNK = 4  # blocks per input-DMA slice

    with tile.TileContext(nc) as tc, ExitStack() as ctx:
        consts = ctx.enter_context(tc.tile_pool(name="consts", bufs=1))
        fpool = ctx.enter_context(tc.tile_pool(name="feat", bufs=8))
        gpool = ctx.enter_context(tc.tile_pool(name="gscratch", bufs=4))
        pspool = ctx.enter_context(tc.tile_pool(name="ps", bufs=2, space="PSUM"))

        # T matrices split per 512-col group so early blocks aren't gated
        # on the full load; xb/tb loads are skipped when no block needs them.
        # DMAs are emitted in consumption order (A and B interleaved).
        b_blocks = {i for i, l in enumerate(schedule) if l > _CA}
        nchunk = -(-nblocks // DMA_CHUNK)

        ta_t = [None, None, None]
        tb_t = [None, None, None] if b_blocks else None
        tcols = _chunks512(_NCOLS)

        def load_t(tiles, dram, name, idx):
            o0, on = tcols[idx]
            t = consts.tile([_KROWS, on], f32r, tag=f"{name}{o0}", name=f"{name}_t{o0}")
            nc.sync.dma_start(out=t, in_=dram[:, o0:o0 + on])
            tiles[idx] = t

        xa_t, xb_t = [None] * nchunk, [None] * nchunk

        def load_xa(ci):
            w0 = ci * DMA_CHUNK * _BLK
            w1 = min(nwords, (ci + 1) * DMA_CHUNK * _BLK)
            xa_t[ci] = consts.tile([_KROWS, w1 - w0], f32r, tag=f"xa{ci}", name=f"xa_t{ci}")
            nc.sync.dma_start(out=xa_t[ci], in_=xa_d[:, w0:w1])

        def load_xb(ci):
            if not any(b in b_blocks for b in range(ci * DMA_CHUNK, (ci + 1) * DMA_CHUNK)):
                return
            w0 = ci * DMA_CHUNK * _BLK
            w1 = min(nwords, (ci + 1) * DMA_CHUNK * _BLK)
            xb_t[ci] = consts.tile([_KROWS, w1 - w0], f32r, tag=f"xb{ci}", name=f"xb_t{ci}")
            nc.sync.dma_start(out=xb_t[ci], in_=xb_d[:, w0:w1])

        # the first (warm-up) blocks only touch ta chunk 0 + xa chunk 0
        load_t(ta_t, ta_d, "ta", 0)
        load_xa(0)
        load_t(ta_t, ta_d, "ta", 1)
        load_t(ta_t, ta_d, "ta", 2)
        if b_blocks:
            for i in range(3):
                load_t(tb_t, tb_d, "tb", i)
            load_xb(0)
        for ci in range(1, nchunk):
            load_xa(ci)
            load_xb(ci)

        def lhs_slice(tiles, b):
            ci, off = divmod(b, DMA_CHUNK)
            return tiles[ci][:, off * _BLK:(off + 1) * _BLK]

        # batch trailing small blocks' outputs into 4-block strips, each
        # DMA'd as soon as its strip completes (cuts HWDGE issue serialization
        # without one big serial end-DMA)
        STRIP = 2
        tail_n = 0
        while (tail_n < 16 and tail_n < nblocks - 1
               and schedule[nblocks - 1 - tail_n] <= 6):
            tail_n += 1
        tail_n -= tail_n % STRIP
        tail_start = nblocks - tail_n
        strips = {}
        if tail_n >= STRIP:
            for si in range(tail_n // STRIP):
                strips[si] = fpool.tile(
                    [_BLK, STRIP * _NOUT], f32, tag="strip", name=f"strip{si}")

        for b in range(nblocks):
            lmax = max(1, min(_C, schedule[b]))
            la = min(lmax, _CA)
            lb = lmax - la
            in_tail = strips and b >= tail_start
            if in_tail:
                si, sj = divmod(b - tail_start, STRIP)
                dst = strips[si][:, sj * _NOUT:(sj + 1) * _NOUT]
            else:
                dst = fpool.tile([_BLK, _NOUT], f32, tag="fm")

            def do_seg(x_tiles, t_tl, lseg, out_ap):
                ncols = lseg * _NOUT
                if ncols <= 512:
                    ps = pspool.tile([_BLK, 512], f32, tag="pss", name="pss")
                else:
                    ps = pspool.tile([_BLK, _NCOLS], f32, tag="ps", name="ps")
                lhs = lhs_slice(x_tiles, b)
                for o0, on in _chunks512(ncols):
                    nc.tensor.matmul(
                        ps[:, o0:o0 + on], lhs, t_tl[o0 // 512][:, 0:on],
                        start=True, stop=True,
                    )
                nc.vector.tensor_reduce(
                    out_ap,
                    ps[:, 0:ncols].rearrange("p (c o) -> p o c", o=_NOUT),
                    axis=X, op=MAX,
                )

            if lb == 0:
                do_seg(xa_t, ta_t, la, dst)
            else:
                fa = fpool.tile([_BLK, _NOUT], f32, tag="f", name="fa")
                fb = fpool.tile([_BLK, _NOUT], f32, tag="f", name="fb")
                do_seg(xa_t, ta_t, la, fa)
                do_seg(xb_t, tb_t, lb, fb)
                nc.vector.tensor_max(dst, fa, fb)

            if in_tail:
                if sj == STRIP - 1:
                    g0 = tail_start + si * STRIP
                    nc.sync.dma_start(
                        out=feat_d[g0 * _BLK:(g0 + STRIP) * _BLK, :].rearrange(
                            "(t r) o -> r t o", r=_BLK),
                        in_=strips[si][:].rearrange("p (t o) -> p t o", o=_NOUT),
                    )
            else:
                nc.sync.dma_start(out=feat_d[b * _BLK:(b + 1) * _BLK, :], in_=dst)

    nc.compile()
    _programs[key] = nc
    return nc


def kernel(**inputs):
    from concourse import bass_utils

    wc = np.asarray(inputs["words_chars"])
    wm = np.asarray(inputs["words_mask"]).astype(bool)
    wcm = np.asarray(inputs["words_chars_mask"]).astype(bool)
    wid = np.asarray(inputs["words_id"])
    emb = np.asarray(inputs["emb"], np.float32)
    ws = {k: np.asarray(inputs[f"w{k}"], np.float32) for k in _KS}
    bs = {k: np.asarray(inputs[f"b{k}"], np.float32) for k in _KS}

    B, W = wm.shape
    C = wc.shape[2]
    assert C == _C
    N = B * W
    flat_mask = wm.reshape(N)
    # Stable valid-first order: matches torch's boolean-mask compaction.
    order = np.argsort(~flat_mask, kind="stable")
    n_valid = int(flat_mask.sum())
    n_needed = max(n_valid, int(wid.max()) + 1, 1)
    stripe = _NCORES * _BLK
    n_pad = -(-n_needed // stripe) * stripe
    nblocks = n_pad // stripe            # per-core block count

    sel = order[:min(n_needed, N)]
    chars = wc.reshape(N, C)[sel].astype(np.int64)
    cmask = wcm.reshape(N, C)[sel]
    if n_pad > len(sel):
        extra = n_pad - len(sel)
        chars = np.concatenate([chars, np.zeros((extra, C), np.int64)], axis=0)
        pmask = np.zeros((extra, C), bool)
        pmask[:, 0] = True               # pad words: L=1, fully masked anyway
        cmask = np.concatenate([cmask, pmask], axis=0)

    # needed position count per word: last valid position + 1 (>=1)
    any_valid = cmask.any(axis=1)
    lastpos = C - 1 - np.argmax(cmask[:, ::-1], axis=1)
    L = np.where(any_valid, lastpos + 1, 1).astype(np.int64)

    # sort by L descending: long blocks first so input DMA hides under their
    # large reduces and the kernel tail is cheap. A few smallest stripes are
    # rotated to the front as cheap pipeline warm-up.
    sort_idx = np.argsort(-L, kind="stable")
    nb_tmp = n_pad // stripe
    rot = min(3, max(0, nb_tmp - 1))
    stripe_order = np.r_[np.arange(nb_tmp - rot, nb_tmp), np.arange(0, nb_tmp - rot)]
    word_perm = (stripe_order[:, None] * stripe + np.arange(stripe)[None, :]).reshape(-1)
    sort_idx = sort_idx[word_perm]
    chars = chars[sort_idx]
    cmask = cmask[sort_idx]
    Ls = L[sort_idx]

    # per-stripe (8 global blocks) Lmax -> identical schedule on every core
    schedule = tuple(
        int(Ls[j * stripe:(j + 1) * stripe].max()) for j in range(nblocks)
    )

    # stripe global blocks to cores: core s, local block j <- global j*8+s
    g_order = np.arange(n_pad).reshape(nblocks, _NCORES, _BLK)
    core_rows = [g_order[:, s, :].reshape(-1) for s in range(_NCORES)]

    ta, tb = _build_toeplitz(ws, bs)
    in_maps = []
    for s in range(_NCORES):
        rows = core_rows[s]
        xa, xb = _build_x(chars[rows], cmask[rows], emb)
        in_maps.append({"xa": xa, "xb": xb, "ta": ta, "tb": tb})

    nc = _get_program(schedule)
    global _last_run
    _last_run = (nc, in_maps)
    res = bass_utils.run_bass_kernel_spmd(nc, in_maps, core_ids=list(range(_NCORES)))

    feats_sorted = np.empty((n_pad, _NOUT), np.float32)
    for s in range(_NCORES):
        feats_sorted[core_rows[s]] = np.asarray(res.results[s]["feat"])
    # un-sort back to compacted order
    feats = np.empty((n_pad, _NOUT), np.float32)
    feats[sort_idx] = feats_sorted
    out = feats[wid.reshape(-1)].reshape(B, W, _NOUT)
    return np.ascontiguousarray(out.astype(np.float32))

